# revision 31
# baseline (speedup 1.0000x reference)
"""DiscretizedMixLogisticLoss Bass kernel for TRN2, 8-core data-parallel. v3.

Full inputs: x [8,3,256,256] f32, l [8,120,256,256] f32 -> nll [8,3,256,256] f32.
Sharding: batch dim N=8 across 8 cores (1 example/core).

Math (per pixel, channel c, mixture k), with l viewed as [4,3,10,H*W]:
  s = l[0], mu = l[1], sc = l[2], co = l[3]
  sig3 = sigmoid(co); m = clip(mu + coupling(sig3*x), 0, 255)
  u = exp(-sc)/2 ; pre' = (m + 0.5 - x)  [negated center]
  f = pre'*u = -t_lo/2 ; g = f - u = -t_hi/2
  2d = tanh(f) - tanh(g)  [mid zone]
The reference computes sigmoid(t) = 1/(1+exp(-t)) in fp32, which keeps full
RELATIVE precision for tiny sigmoids; tanh differences lose it.  Two far-zone
patches replicate the reference's fp32 behavior:
  x<<m (g>=4):  2d = 2(e^{t_hi} - e^{t_lo})          [full rel precision]
  x>>m (f<=-6): 2d = 2[(1+e^{-t_lo}) - (1+e^{-t_hi})] [replicates the
                reference's near-1.0 quantization grid exactly]
Both come from one exp of -2|f|,-2|g| (+ln2 predoubling). Then
  e1 = exp(s) ; e2 = max(2d, 2e-12)*e1
  nll = ln(s1) - ln(0.5*s2),  s1 = sum_k e1, s2 = sum_k e2.
Edge pixels (x<0.001 / x>254.999, ~13 of 1.5M) are fixed up on host.

Layout per core: partition p = (c*10+k)*4 + q (q = 16384-pixel quarter),
free dim = J=2048 pixels per tile, 8 tiles.  lg/mu/co staged as fp16 (sc must
stay fp32 - it scales log-d multiplicatively).  PE does x broadcasts, the RGB
coupling sum, the mu add, and K-sums (fp32r = tf32-grade 1-pass; bf16 e1/e2).
ACT: all transcendentals (exp/tanh/abs, one table set) + deferred Ln tails.
DVE/GPSIMD split the elementwise chain.
"""
from contextlib import ExitStack

import numpy as np
import ml_dtypes

import concourse.bass as bass
import concourse.bacc as bacc
import concourse.tile as tile
from concourse import mybir
from concourse.bass_utils import run_bass_kernel_spmd

AF = mybir.ActivationFunctionType
ALU = mybir.AluOpType
F32 = mybir.dt.float32
F32R = mybir.dt.float32r
F16 = mybir.dt.float16
BF16 = mybir.dt.bfloat16
U8 = mybir.dt.uint8

N, C, K, H, W = 8, 3, 10, 256, 256
HW = H * W            # 65536 pixels per example
CK = C * K            # 30
P = CK * 4            # 120 partitions used
NCORES = 8

J = 1024              # pixels per partition per tile
NT = HW // (4 * J)    # 16 tiles per core
QS = HW // 4          # 16384 pixel-quarter stride
LN2 = 0.6931471805599453
THN = float(np.tanh(4.0))    # neg-zone mask threshold on tanh(g)
THP = float(np.tanh(-6.0))   # pos-zone mask threshold on tanh(f)


def _param_ap(t16, t):
    """DRAM AP over a [CK, HW] param tensor matching SBUF [120, J] with
    partition p = ck*4+q, pixel q*QS + t*J + j."""
    return bass.AP(tensor=t16, offset=t * J,
                   ap=[[HW, CK], [QS, 4], [1, J]])


def _x_ap(xt, t):
    return bass.AP(tensor=xt, offset=t * J,
                   ap=[[HW, C], [QS, 4], [1, J]])


def _out_ap(out, t):
    return bass.AP(tensor=out, offset=t * J,
                   ap=[[HW, C], [QS, 4], [1, J]])


def build_kernel():
    nc = bacc.Bacc("TRN2", target_bir_lowering=False, debug=False)

    lg_d = nc.dram_tensor("lg16", [CK, HW], F16, kind="ExternalInput")
    mu_d = nc.dram_tensor("mu16", [CK, HW], F16, kind="ExternalInput")
    sc_d = nc.dram_tensor("sc32", [CK, HW], F32, kind="ExternalInput")
    co_d = nc.dram_tensor("co16", [CK, HW], F16, kind="ExternalInput")
    x_d = nc.dram_tensor("x32", [C, HW], F32R, kind="ExternalInput")
    wr_d = nc.dram_tensor("w32r", [P, 360], F32R, kind="ExternalInput")
    wi_d = nc.dram_tensor("w16i", [P, P], F16, kind="ExternalInput")
    ws_d = nc.dram_tensor("wbfs", [P, 12], BF16, kind="ExternalInput")
    out = nc.dram_tensor("out", [C, HW], F32, kind="ExternalOutput")

    with tile.TileContext(nc) as tc, ExitStack() as ctx:
        consts = ctx.enter_context(tc.tile_pool(name="consts", bufs=1))
        lpool = ctx.enter_context(tc.tile_pool(name="lpool", bufs=4))
        w2 = ctx.enter_context(tc.tile_pool(name="w2", bufs=3))
        w1 = ctx.enter_context(tc.tile_pool(name="w1", bufs=3))
        psum = ctx.enter_context(tc.tile_pool(name="psum", bufs=2, space="PSUM"))
        stand = ctx.enter_context(tc.tile_pool(name="stand", bufs=1))

        wr = consts.tile([P, 360], F32R)
        wi = consts.tile([P, P], F16)
        ws = consts.tile([P, 12], BF16)
        nc.sync.dma_start(out=wr, in_=wr_d[:, :])
        nc.sync.dma_start(out=wi, in_=wi_d[:, :])
        nc.sync.dma_start(out=ws, in_=ws_d[:, :])
        w_coup = wr[:, 0:120]
        w_cxb = wr[0:12, 120:240]
        w_xb = wr[0:12, 240:360]
        bln2n = consts.tile([P, 1], F32)
        nc.vector.memset(bln2n, -LN2)
        bln2p = consts.tile([P, 1], F32)
        nc.vector.memset(bln2p, LN2)
        # standing results, two groups so group A's ln-tail overlaps group B.
        # partition = (t - t0)*12 + (c*4+q); cols [0:J]=s1, [J:2J]=s2.
        NTA = NT // 2
        rbufA = stand.tile([NTA * 12, 2 * J], F32)
        rbufB = stand.tile([(NT - NTA) * 12, 2 * J], F32)

        def _tail(rb, t0, nt):
            nc.scalar.activation(out=rb[:, 0:J], in_=rb[:, 0:J], func=AF.Ln)
            nc.scalar.activation(out=rb[:, J:2 * J], in_=rb[:, J:2 * J],
                                 func=AF.Ln, scale=0.5)
            nc.vector.tensor_tensor(out=rb[:, 0:J], in0=rb[:, 0:J],
                                    in1=rb[:, J:2 * J], op=ALU.subtract)
            for tt_ in range(nt):
                nc.scalar.dma_start(out=_out_ap(out, t0 + tt_),
                                    in_=rb[tt_ * 12:(tt_ + 1) * 12, 0:J])

        def front(t):
            lg = lpool.tile([P, J], F16, tag="lg")
            mu = lpool.tile([P, J], F16, tag="mu")
            sc = lpool.tile([P, J], F32, tag="sc")
            co = lpool.tile([P, J], F16, tag="co")
            nc.sync.dma_start(out=lg, in_=_param_ap(lg_d, t))
            nc.scalar.dma_start(out=mu, in_=_param_ap(mu_d, t))
            nc.scalar.dma_start(out=sc, in_=_param_ap(sc_d, t))
            nc.sync.dma_start(out=co, in_=_param_ap(co_d, t))
            xq = w1.tile([12, J], F32R, tag="xq")
            nc.sync.dma_start(out=xq, in_=_x_ap(x_d, t))

            # ACT transcendentals (exp_and_others set: exp + tanh + abs)
            th = w1.tile([P, J], F32R, tag="th")
            nc.scalar.activation(out=th, in_=co, func=AF.Tanh, scale=0.5)
            u = w1.tile([P, J], F32, tag="u")
            nc.scalar.activation(out=u, in_=sc, func=AF.Exp, scale=-1.0,
                                 bias=bln2n)
            e1 = w2.tile([P, J], BF16, tag="e1")
            nc.scalar.activation(out=e1, in_=lg, func=AF.Exp)

            # PE: cxb = 0.5 * x[chan(cc)] broadcast over k (fp32r, 1-pass)
            cxb = psum.tile([P, J], F32, tag="pa")
            for i in range(J // 512):
                s0, s1 = i * 512, (i + 1) * 512
                nc.tensor.matmul(cxb[:, s0:s1], w_cxb, xq[:, s0:s1],
                                 start=True, stop=True)
            # t1 = sigmoid(co)*cx = (th+1)*cxb   (in place over th)
            nc.vector.scalar_tensor_tensor(out=th, in0=th, scalar=1.0,
                                           in1=cxb, op0=ALU.add, op1=ALU.mult)
            # PE: m = mu + coupling  (same psum slot as cxb)
            m = psum.tile([P, J], F32, tag="pa")
            for i in range(J // 512):
                s0, s1 = i * 512, (i + 1) * 512
                nc.tensor.matmul(m[:, s0:s1], wi, mu[:, s0:s1],
                                 start=True, stop=False)
                nc.tensor.matmul(m[:, s0:s1], w_coup, th[:, s0:s1],
                                 start=False, stop=True)
            # cm = clip(m, 0, 255)
            cm = w1.tile([P, J], F32, tag="cm")
            nc.vector.tensor_scalar(out=cm, in0=m, scalar1=0.0, scalar2=255.0,
                                    op0=ALU.max, op1=ALU.min)
            # PE: xb = x[c] broadcast over k
            xb = psum.tile([P, J], F32, tag="pb")
            for i in range(J // 512):
                s0, s1 = i * 512, (i + 1) * 512
                nc.tensor.matmul(xb[:, s0:s1], w_xb, xq[:, s0:s1],
                                 start=True, stop=True)
            # pre' = (cm + 0.5) - x   (in place over cm)
            nc.vector.scalar_tensor_tensor(out=cm, in0=cm, scalar=0.5,
                                           in1=xb, op0=ALU.add,
                                           op1=ALU.subtract)
            # f = pre'*u ; g = f - u   (halves of ab)
            ab = w2.tile([P, 2 * J], F32, tag="ab")
            nc.gpsimd.tensor_tensor(out=ab[:, 0:J], in0=cm, in1=u,
                                    op=ALU.mult)
            nc.gpsimd.tensor_tensor(out=ab[:, J:2 * J], in0=ab[:, 0:J],
                                    in1=u, op=ALU.subtract)
            # exps = 2*exp(-2|ab|) = [E2|F2] ; then tanh(ab) in place
            exps = w2.tile([P, 2 * J], F32, tag="exps")
            nc.scalar.activation(out=exps, in_=ab, func=AF.Abs)
            nc.scalar.activation(out=ab, in_=ab, func=AF.Tanh)
            nc.scalar.activation(out=exps, in_=exps, func=AF.Exp, scale=-2.0,
                                 bias=bln2p)
            # masks BEFORE dsub overwrites tanh(f)
            mkn = w1.tile([P, J], U8, tag="mkn")
            nc.vector.tensor_scalar(out=mkn, in0=ab[:, J:2 * J], scalar1=THN,
                                    scalar2=None, op0=ALU.is_ge)
            mkp = w1.tile([P, J], U8, tag="mkp")
            nc.vector.tensor_scalar(out=mkp, in0=ab[:, 0:J], scalar1=THP,
                                    scalar2=None, op0=ALU.is_le)
            # dneg = F2 - E2 (into cm slot, dead after f/g)
            nc.gpsimd.tensor_tensor(out=cm, in0=exps[:, J:2 * J],
                                    in1=exps[:, 0:J], op=ALU.subtract)
            # dsub2 = tanh(f) - tanh(g)   (in place over ab[:,0:J])
            nc.gpsimd.tensor_tensor(out=ab[:, 0:J], in0=ab[:, 0:J],
                                    in1=ab[:, J:2 * J], op=ALU.subtract)
            nc.vector.copy_predicated(out=ab[:, 0:J], mask=mkn, data=cm)
            # pos-grid: P2 = 2 + exps = 2*(1 + e^{-t}); the fp32 grid at
            # [2,4) is exactly 2x the grid at [1,2), so P2_f - P2_g equals
            # 2*[fl(1+e^{-t_lo}) - fl(1+e^{-t_hi})] = the reference's d (x2).
            nc.vector.tensor_scalar(out=exps, in0=exps, scalar1=2.0,
                                    scalar2=None, op0=ALU.add)
            dpos = w1.tile([P, J], F32, tag="dpos")
            nc.vector.tensor_tensor(out=dpos, in0=exps[:, 0:J],
                                    in1=exps[:, J:2 * J], op=ALU.subtract)
            nc.vector.copy_predicated(out=ab[:, 0:J], mask=mkp, data=dpos)
            # e2 = max(2d, 2e-12) * e1   (bf16 out for 1-pass K-sum)
            e2 = w2.tile([P, J], BF16, tag="e2")
            nc.vector.scalar_tensor_tensor(out=e2, in0=ab[:, 0:J],
                                           scalar=2e-12, in1=e1,
                                           op0=ALU.max, op1=ALU.mult)
            return dict(t=t, e1=e1, e2=e2)

        def ksum_store(st):
            t, e1, e2 = st["t"], st["e1"], st["e2"]
            # PE K-sums into PSUM (bf16, 1-pass): r1 @ 0, r2 @ 32
            rp = psum.tile([44, J], F32, tag="pb")
            for i in range(J // 512):
                s0, s1 = i * 512, (i + 1) * 512
                nc.tensor.matmul(rp[0:12, s0:s1], ws, e1[:, s0:s1],
                                 start=True, stop=True)
                nc.tensor.matmul(rp[32:44, s0:s1], ws, e2[:, s0:s1],
                                 start=True, stop=True)
            # PSUM -> SBUF scratch -> standing rbuf slice
            rsc = w1.tile([44, J], F32, tag="rsc")
            nc.scalar.copy(out=rsc, in_=rp)
            rb, tb = (rbufA, t) if t < NTA else (rbufB, t - NTA)
            nc.gpsimd.dma_start(out=rb[tb * 12:(tb + 1) * 12, 0:J],
                                in_=rsc[0:12, :])
            nc.gpsimd.dma_start(out=rb[tb * 12:(tb + 1) * 12, J:2 * J],
                                in_=rsc[32:44, :])
            if t == NTA - 1:
                _tail(rbufA, 0, NTA)

        # K-sums/stores run one tile behind so the PE stream never parks on
        # a not-yet-ready e2 in front of the next tile's broadcasts.
        prev = None
        for t in range(NT):
            st = front(t)
            if prev is not None:
                ksum_store(prev)
            prev = st
        ksum_store(prev)
        _tail(rbufB, NTA, NT - NTA)

    nc.compile()
    return nc


_CONSTS = None
_NC_CACHE = None


def _consts_np():
    global _CONSTS
    if _CONSTS is None:
        chan = {0: 0, 1: 0, 2: 1}   # coupling x-source channel per cc
        ccc = {0: 1, 1: 2, 2: 2}    # coupling target channel per cc
        wr = np.zeros((P, 360), dtype=np.float32)
        for cc in range(3):
            for k in range(K):
                for q in range(4):
                    pin = (cc * K + k) * 4 + q
                    wr[pin, ccc[cc] * K * 4 + k * 4 + q] = 1.0       # w_coup
                    wr[chan[cc] * 4 + q, 120 + pin] = 0.5            # w_cxb
        for c in range(C):
            for k in range(K):
                for q in range(4):
                    pin = (c * K + k) * 4 + q
                    wr[c * 4 + q, 240 + pin] = 1.0                   # w_xb
        wi = np.eye(P, dtype=np.float16)
        ws = np.zeros((P, 12), dtype=ml_dtypes.bfloat16)
        for c in range(C):
            for k in range(K):
                for q in range(4):
                    ws[(c * K + k) * 4 + q, c * 4 + q] = 1.0         # K-sum
        _CONSTS = (wr, wi, ws)
    return _CONSTS


def _host_fixup(nll, x, l):
    """Recompute edge pixels (lo_cond/hi_cond active) exactly on host."""
    f32 = np.float32
    mask = (x < f32(0.001)) | (x > f32(254.999))
    if not mask.any():
        return nll
    l6 = l.reshape(N, 4, C, K, H, W)
    with np.errstate(over="ignore"):
        sg = lambda z: (f32(1) / (f32(1) + np.exp(-z, dtype=f32))).astype(f32)
        for n, cc, hh, ww in zip(*np.nonzero(mask)):
            s = l6[n, 0, cc, :, hh, ww]
            m_raw = l6[n, 1, :, :, hh, ww]
            sc_ = np.maximum(l6[n, 2, cc, :, hh, ww], f32(-7))
            co = sg(l6[n, 3, :, :, hh, ww])
            xpix = x[n, :, hh, ww]
            if cc == 0:
                m = m_raw[0]
            elif cc == 1:
                m = (m_raw[1] + co[0] * xpix[0]).astype(f32)
            else:
                m = (m_raw[2] + co[1] * xpix[0] + co[2] * xpix[1]).astype(f32)
            m = np.clip(m, f32(0), f32(255)).astype(f32)
            cen = (xpix[cc] - m).astype(f32)
            invv = np.exp(-sc_, dtype=f32)
            lo_c = f32(1) if xpix[cc] >= f32(0.001) else f32(0)
            hi_c = f32(1) if xpix[cc] <= f32(254.999) else f32(0)
            cdf_lo = lo_c * sg(invv * (cen - f32(0.5)))
            cdf_hi = hi_c * sg(invv * (cen + f32(0.5))) + (f32(1) - hi_c)
            d = np.maximum(cdf_hi - cdf_lo, f32(1e-12))
            e1 = np.exp(s, dtype=f32)
            e2 = (e1 * d).astype(f32)
            nll[n, cc, hh, ww] = np.log(e1.sum(dtype=f32), dtype=f32) - np.log(
                e2.sum(dtype=f32), dtype=f32)
    return nll


def _get_nc():
    global _NC_CACHE
    if _NC_CACHE is None:
        _NC_CACHE = build_kernel()
    return _NC_CACHE


def _make_in_maps(x, l):
    wr, wi, ws = _consts_np()
    l6 = l.reshape(N, 4, CK, HW)
    lg16 = l6[:, 0].astype(np.float16)
    mu16 = l6[:, 1].astype(np.float16)
    sc32 = np.ascontiguousarray(l6[:, 2])
    co16 = l6[:, 3].astype(np.float16)
    x2 = x.reshape(N, C, HW)
    return [
        {"lg16": lg16[n], "mu16": mu16[n], "sc32": sc32[n], "co16": co16[n],
         "x32": np.ascontiguousarray(x2[n]),
         "w32r": wr, "w16i": wi, "wbfs": ws}
        for n in range(NCORES)
    ]


def kernel(x, l):
    x = np.ascontiguousarray(x, dtype=np.float32)
    l = np.ascontiguousarray(l, dtype=np.float32)
    nc = _get_nc()
    in_maps = _make_in_maps(x, l)
    res = run_bass_kernel_spmd(nc, in_maps, list(range(NCORES))).results
    nll = np.stack([res[n]["out"].reshape(C, H, W) for n in range(NCORES)],
                   axis=0)
    return _host_fixup(nll, x, l)


# revision 33
# speedup vs baseline: 1.0394x; 1.0394x over previous
"""DiscretizedMixLogisticLoss Bass kernel for TRN2, 8-core data-parallel. v3.

Full inputs: x [8,3,256,256] f32, l [8,120,256,256] f32 -> nll [8,3,256,256] f32.
Sharding: batch dim N=8 across 8 cores (1 example/core).

Math (per pixel, channel c, mixture k), with l viewed as [4,3,10,H*W]:
  s = l[0], mu = l[1], sc = l[2], co = l[3]
  sig3 = sigmoid(co); m = clip(mu + coupling(sig3*x), 0, 255)
  u = exp(-sc)/2 ; pre' = (m + 0.5 - x)  [negated center]
  f = pre'*u = -t_lo/2 ; g = f - u = -t_hi/2
  2d = tanh(f) - tanh(g)  [mid zone]
The reference computes sigmoid(t) = 1/(1+exp(-t)) in fp32, which keeps full
RELATIVE precision for tiny sigmoids; tanh differences lose it.  Two far-zone
patches replicate the reference's fp32 behavior:
  x<<m (g>=4):  2d = 2(e^{t_hi} - e^{t_lo})          [full rel precision]
  x>>m (f<=-6): 2d = 2[(1+e^{-t_lo}) - (1+e^{-t_hi})] [replicates the
                reference's near-1.0 quantization grid exactly]
Both come from one exp of -2|f|,-2|g| (+ln2 predoubling). Then
  e1 = exp(s) ; e2 = max(2d, 2e-12)*e1
  nll = ln(s1) - ln(0.5*s2),  s1 = sum_k e1, s2 = sum_k e2.
Edge pixels (x<0.001 / x>254.999, ~13 of 1.5M) are fixed up on host.

Layout per core: partition p = (c*10+k)*4 + q (q = 16384-pixel quarter),
free dim = J=2048 pixels per tile, 8 tiles.  lg/mu/co staged as fp16 (sc must
stay fp32 - it scales log-d multiplicatively).  PE does x broadcasts, the RGB
coupling sum, the mu add, and K-sums (fp32r = tf32-grade 1-pass; bf16 e1/e2).
ACT: all transcendentals (exp/tanh/abs, one table set) + deferred Ln tails.
DVE/GPSIMD split the elementwise chain.
"""
from contextlib import ExitStack

import numpy as np
import ml_dtypes

import concourse.bass as bass
import concourse.bacc as bacc
import concourse.tile as tile
from concourse import mybir
from concourse.bass_utils import run_bass_kernel_spmd

AF = mybir.ActivationFunctionType
ALU = mybir.AluOpType
F32 = mybir.dt.float32
F32R = mybir.dt.float32r
F16 = mybir.dt.float16
BF16 = mybir.dt.bfloat16
U8 = mybir.dt.uint8

N, C, K, H, W = 8, 3, 10, 256, 256
HW = H * W            # 65536 pixels per example
CK = C * K            # 30
P = CK * 4            # 120 partitions used
NCORES = 8

J = 1024              # pixels per partition per tile
NT = HW // (4 * J)    # 16 tiles per core
QS = HW // 4          # 16384 pixel-quarter stride
LN2 = 0.6931471805599453
THN = float(np.tanh(4.0))    # neg-zone mask threshold on tanh(g)
THP = float(np.tanh(-6.0))   # pos-zone mask threshold on tanh(f)


def _param_ap(t16, t):
    """DRAM AP over a [CK, HW] param tensor matching SBUF [120, J] with
    partition p = ck*4+q, pixel q*QS + t*J + j."""
    return bass.AP(tensor=t16, offset=t * J,
                   ap=[[HW, CK], [QS, 4], [1, J]])


def _x_ap(xt, t):
    return bass.AP(tensor=xt, offset=t * J,
                   ap=[[HW, C], [QS, 4], [1, J]])


def _out_ap(out, t):
    return bass.AP(tensor=out, offset=t * J,
                   ap=[[HW, C], [QS, 4], [1, J]])


def build_kernel():
    nc = bacc.Bacc("TRN2", target_bir_lowering=False, debug=False)

    lg_d = nc.dram_tensor("lg16", [CK, HW], F16, kind="ExternalInput")
    mu_d = nc.dram_tensor("mu16", [CK, HW], F16, kind="ExternalInput")
    sc_d = nc.dram_tensor("sc32", [CK, HW], F32, kind="ExternalInput")
    co_d = nc.dram_tensor("co16", [CK, HW], F16, kind="ExternalInput")
    x_d = nc.dram_tensor("x32", [C, HW], F32R, kind="ExternalInput")
    wr_d = nc.dram_tensor("w32r", [P, 360], F32R, kind="ExternalInput")
    wi_d = nc.dram_tensor("w16i", [P, P], F16, kind="ExternalInput")
    ws_d = nc.dram_tensor("wbfs", [P, 12], BF16, kind="ExternalInput")
    out = nc.dram_tensor("out", [C, HW], F32, kind="ExternalOutput")

    with tile.TileContext(nc) as tc, ExitStack() as ctx:
        consts = ctx.enter_context(tc.tile_pool(name="consts", bufs=1))
        lpool = ctx.enter_context(tc.tile_pool(name="lpool", bufs=4))
        w2 = ctx.enter_context(tc.tile_pool(name="w2", bufs=3))
        w1 = ctx.enter_context(tc.tile_pool(name="w1", bufs=3))
        psum = ctx.enter_context(tc.tile_pool(name="psum", bufs=2, space="PSUM"))
        stand = ctx.enter_context(tc.tile_pool(name="stand", bufs=1))

        wr = consts.tile([P, 360], F32R)
        wi = consts.tile([P, P], F16)
        ws = consts.tile([P, 12], BF16)
        nc.sync.dma_start(out=wr, in_=wr_d[:, :])
        nc.sync.dma_start(out=wi, in_=wi_d[:, :])
        nc.sync.dma_start(out=ws, in_=ws_d[:, :])
        w_coup = wr[:, 0:120]
        w_cxb = wr[0:12, 120:240]
        w_xb = wr[0:12, 240:360]
        bln2n = consts.tile([P, 1], F32)
        nc.vector.memset(bln2n, -LN2)
        bln2p = consts.tile([P, 1], F32)
        nc.vector.memset(bln2p, LN2)
        # standing results, two groups so group A's ln-tail overlaps group B.
        # partition = (t - t0)*12 + (c*4+q); cols [0:J]=s1, [J:2J]=s2.
        NTA = NT // 2
        rbufA = stand.tile([NTA * 12, 2 * J], F32)
        rbufB = stand.tile([(NT - NTA) * 12, 2 * J], F32)

        def _tail(rb, t0, nt):
            nc.scalar.activation(out=rb[:, 0:J], in_=rb[:, 0:J], func=AF.Ln)
            nc.scalar.activation(out=rb[:, J:2 * J], in_=rb[:, J:2 * J],
                                 func=AF.Ln, scale=0.5)
            nc.vector.tensor_tensor(out=rb[:, 0:J], in0=rb[:, 0:J],
                                    in1=rb[:, J:2 * J], op=ALU.subtract)
            for tt_ in range(nt):
                nc.sync.dma_start(out=_out_ap(out, t0 + tt_),
                                  in_=rb[tt_ * 12:(tt_ + 1) * 12, 0:J])

        def front(t):
            lg = lpool.tile([P, J], F16, tag="lg")
            mu = lpool.tile([P, J], F16, tag="mu")
            sc = lpool.tile([P, J], F32, tag="sc")
            co = lpool.tile([P, J], F16, tag="co")
            nc.sync.dma_start(out=lg, in_=_param_ap(lg_d, t))
            nc.scalar.dma_start(out=mu, in_=_param_ap(mu_d, t))
            nc.scalar.dma_start(out=sc, in_=_param_ap(sc_d, t))
            nc.sync.dma_start(out=co, in_=_param_ap(co_d, t))
            xq = w1.tile([12, J], F32R, tag="xq")
            nc.sync.dma_start(out=xq, in_=_x_ap(x_d, t))

            # ACT transcendentals (exp_and_others set: exp + tanh + abs)
            th = w1.tile([P, J], F32R, tag="th")
            nc.scalar.activation(out=th, in_=co, func=AF.Tanh, scale=0.5)
            u = w1.tile([P, J], F32, tag="u")
            nc.scalar.activation(out=u, in_=sc, func=AF.Exp, scale=-1.0,
                                 bias=bln2n)
            e1 = w2.tile([P, J], BF16, tag="e1")
            nc.scalar.activation(out=e1, in_=lg, func=AF.Exp)

            # PE: cxb = 0.5 * x[chan(cc)] broadcast over k (fp32r, 1-pass)
            cxb = psum.tile([P, J], F32, tag="pa")
            for i in range(J // 512):
                s0, s1 = i * 512, (i + 1) * 512
                nc.tensor.matmul(cxb[:, s0:s1], w_cxb, xq[:, s0:s1],
                                 start=True, stop=True)
            # t1 = sigmoid(co)*cx = (th+1)*cxb   (in place over th)
            nc.vector.scalar_tensor_tensor(out=th, in0=th, scalar=1.0,
                                           in1=cxb, op0=ALU.add, op1=ALU.mult)
            # PE: m = mu + coupling  (same psum slot as cxb)
            m = psum.tile([P, J], F32, tag="pa")
            for i in range(J // 512):
                s0, s1 = i * 512, (i + 1) * 512
                nc.tensor.matmul(m[:, s0:s1], wi, mu[:, s0:s1],
                                 start=True, stop=False)
                nc.tensor.matmul(m[:, s0:s1], w_coup, th[:, s0:s1],
                                 start=False, stop=True)
            # cm = clip(m, 0, 255)
            cm = w1.tile([P, J], F32, tag="cm")
            nc.vector.tensor_scalar(out=cm, in0=m, scalar1=0.0, scalar2=255.0,
                                    op0=ALU.max, op1=ALU.min)
            # PE: xb = x[c] broadcast over k
            xb = psum.tile([P, J], F32, tag="pb")
            for i in range(J // 512):
                s0, s1 = i * 512, (i + 1) * 512
                nc.tensor.matmul(xb[:, s0:s1], w_xb, xq[:, s0:s1],
                                 start=True, stop=True)
            # pre' = (cm + 0.5) - x   (in place over cm)
            nc.vector.scalar_tensor_tensor(out=cm, in0=cm, scalar=0.5,
                                           in1=xb, op0=ALU.add,
                                           op1=ALU.subtract)
            # f = pre'*u ; g = f - u   (halves of ab)
            ab = w2.tile([P, 2 * J], F32, tag="ab")
            nc.gpsimd.tensor_tensor(out=ab[:, 0:J], in0=cm, in1=u,
                                    op=ALU.mult)
            nc.gpsimd.tensor_tensor(out=ab[:, J:2 * J], in0=ab[:, 0:J],
                                    in1=u, op=ALU.subtract)
            # exps = 2*exp(-2|ab|) = [E2|F2] ; then tanh(ab) in place
            exps = w2.tile([P, 2 * J], F32, tag="exps")
            nc.scalar.activation(out=exps, in_=ab, func=AF.Abs)
            nc.scalar.activation(out=exps, in_=exps, func=AF.Exp, scale=-2.0,
                                 bias=bln2p)
            nc.scalar.activation(out=ab, in_=ab, func=AF.Tanh)
            # masks BEFORE dsub overwrites tanh(f)
            mkn = w1.tile([P, J], U8, tag="mkn")
            nc.vector.tensor_scalar(out=mkn, in0=ab[:, J:2 * J], scalar1=THN,
                                    scalar2=None, op0=ALU.is_ge)
            mkp = w1.tile([P, J], U8, tag="mkp")
            nc.vector.tensor_scalar(out=mkp, in0=ab[:, 0:J], scalar1=THP,
                                    scalar2=None, op0=ALU.is_le)
            # dneg = F2 - E2 (into cm slot, dead after f/g)
            nc.gpsimd.tensor_tensor(out=cm, in0=exps[:, J:2 * J],
                                    in1=exps[:, 0:J], op=ALU.subtract)
            # dsub2 = tanh(f) - tanh(g)   (in place over ab[:,0:J])
            nc.gpsimd.tensor_tensor(out=ab[:, 0:J], in0=ab[:, 0:J],
                                    in1=ab[:, J:2 * J], op=ALU.subtract)
            nc.vector.copy_predicated(out=ab[:, 0:J], mask=mkn, data=cm)
            # pos-grid: P2 = 2 + exps = 2*(1 + e^{-t}); the fp32 grid at
            # [2,4) is exactly 2x the grid at [1,2), so P2_f - P2_g equals
            # 2*[fl(1+e^{-t_lo}) - fl(1+e^{-t_hi})] = the reference's d (x2).
            nc.vector.tensor_scalar(out=exps, in0=exps, scalar1=2.0,
                                    scalar2=None, op0=ALU.add)
            dpos = w1.tile([P, J], F32, tag="dpos")
            nc.vector.tensor_tensor(out=dpos, in0=exps[:, 0:J],
                                    in1=exps[:, J:2 * J], op=ALU.subtract)
            nc.vector.copy_predicated(out=ab[:, 0:J], mask=mkp, data=dpos)
            # e2 = max(2d, 2e-12) * e1   (bf16 out for 1-pass K-sum)
            e2 = w2.tile([P, J], BF16, tag="e2")
            nc.vector.scalar_tensor_tensor(out=e2, in0=ab[:, 0:J],
                                           scalar=2e-12, in1=e1,
                                           op0=ALU.max, op1=ALU.mult)
            return dict(t=t, e1=e1, e2=e2)

        def ksum_store(st):
            t, e1, e2 = st["t"], st["e1"], st["e2"]
            # PE K-sums into PSUM (bf16, 1-pass): r1 @ 0, r2 @ 32
            rp = psum.tile([44, J], F32, tag="pb")
            for i in range(J // 512):
                s0, s1 = i * 512, (i + 1) * 512
                nc.tensor.matmul(rp[0:12, s0:s1], ws, e1[:, s0:s1],
                                 start=True, stop=True)
                nc.tensor.matmul(rp[32:44, s0:s1], ws, e2[:, s0:s1],
                                 start=True, stop=True)
            # PSUM -> SBUF scratch -> standing rbuf slice
            rsc = w1.tile([44, J], F32, tag="rsc")
            nc.scalar.copy(out=rsc, in_=rp)
            rb, tb = (rbufA, t) if t < NTA else (rbufB, t - NTA)
            nc.sync.dma_start(out=rb[tb * 12:(tb + 1) * 12, 0:J],
                              in_=rsc[0:12, :])
            nc.sync.dma_start(out=rb[tb * 12:(tb + 1) * 12, J:2 * J],
                              in_=rsc[32:44, :])
            if t == NTA - 1:
                _tail(rbufA, 0, NTA)

        # K-sums/stores run one tile behind so the PE stream never parks on
        # a not-yet-ready e2 in front of the next tile's broadcasts.
        prev = None
        for t in range(NT):
            st = front(t)
            if prev is not None:
                ksum_store(prev)
            prev = st
        ksum_store(prev)
        _tail(rbufB, NTA, NT - NTA)

    nc.compile()
    return nc


_CONSTS = None
_NC_CACHE = None


def _consts_np():
    global _CONSTS
    if _CONSTS is None:
        chan = {0: 0, 1: 0, 2: 1}   # coupling x-source channel per cc
        ccc = {0: 1, 1: 2, 2: 2}    # coupling target channel per cc
        wr = np.zeros((P, 360), dtype=np.float32)
        for cc in range(3):
            for k in range(K):
                for q in range(4):
                    pin = (cc * K + k) * 4 + q
                    wr[pin, ccc[cc] * K * 4 + k * 4 + q] = 1.0       # w_coup
                    wr[chan[cc] * 4 + q, 120 + pin] = 0.5            # w_cxb
        for c in range(C):
            for k in range(K):
                for q in range(4):
                    pin = (c * K + k) * 4 + q
                    wr[c * 4 + q, 240 + pin] = 1.0                   # w_xb
        wi = np.eye(P, dtype=np.float16)
        ws = np.zeros((P, 12), dtype=ml_dtypes.bfloat16)
        for c in range(C):
            for k in range(K):
                for q in range(4):
                    ws[(c * K + k) * 4 + q, c * 4 + q] = 1.0         # K-sum
        _CONSTS = (wr, wi, ws)
    return _CONSTS


def _host_fixup(nll, x, l):
    """Recompute edge pixels (lo_cond/hi_cond active) exactly on host."""
    f32 = np.float32
    mask = (x < f32(0.001)) | (x > f32(254.999))
    if not mask.any():
        return nll
    l6 = l.reshape(N, 4, C, K, H, W)
    with np.errstate(over="ignore"):
        sg = lambda z: (f32(1) / (f32(1) + np.exp(-z, dtype=f32))).astype(f32)
        for n, cc, hh, ww in zip(*np.nonzero(mask)):
            s = l6[n, 0, cc, :, hh, ww]
            m_raw = l6[n, 1, :, :, hh, ww]
            sc_ = np.maximum(l6[n, 2, cc, :, hh, ww], f32(-7))
            co = sg(l6[n, 3, :, :, hh, ww])
            xpix = x[n, :, hh, ww]
            if cc == 0:
                m = m_raw[0]
            elif cc == 1:
                m = (m_raw[1] + co[0] * xpix[0]).astype(f32)
            else:
                m = (m_raw[2] + co[1] * xpix[0] + co[2] * xpix[1]).astype(f32)
            m = np.clip(m, f32(0), f32(255)).astype(f32)
            cen = (xpix[cc] - m).astype(f32)
            invv = np.exp(-sc_, dtype=f32)
            lo_c = f32(1) if xpix[cc] >= f32(0.001) else f32(0)
            hi_c = f32(1) if xpix[cc] <= f32(254.999) else f32(0)
            cdf_lo = lo_c * sg(invv * (cen - f32(0.5)))
            cdf_hi = hi_c * sg(invv * (cen + f32(0.5))) + (f32(1) - hi_c)
            d = np.maximum(cdf_hi - cdf_lo, f32(1e-12))
            e1 = np.exp(s, dtype=f32)
            e2 = (e1 * d).astype(f32)
            nll[n, cc, hh, ww] = np.log(e1.sum(dtype=f32), dtype=f32) - np.log(
                e2.sum(dtype=f32), dtype=f32)
    return nll


def _get_nc():
    global _NC_CACHE
    if _NC_CACHE is None:
        _NC_CACHE = build_kernel()
    return _NC_CACHE


def _make_in_maps(x, l):
    wr, wi, ws = _consts_np()
    l6 = l.reshape(N, 4, CK, HW)
    lg16 = l6[:, 0].astype(np.float16)
    mu16 = l6[:, 1].astype(np.float16)
    sc32 = np.ascontiguousarray(l6[:, 2])
    co16 = l6[:, 3].astype(np.float16)
    x2 = x.reshape(N, C, HW)
    return [
        {"lg16": lg16[n], "mu16": mu16[n], "sc32": sc32[n], "co16": co16[n],
         "x32": np.ascontiguousarray(x2[n]),
         "w32r": wr, "w16i": wi, "wbfs": ws}
        for n in range(NCORES)
    ]


def kernel(x, l):
    x = np.ascontiguousarray(x, dtype=np.float32)
    l = np.ascontiguousarray(l, dtype=np.float32)
    nc = _get_nc()
    in_maps = _make_in_maps(x, l)
    res = run_bass_kernel_spmd(nc, in_maps, list(range(NCORES))).results
    nll = np.stack([res[n]["out"].reshape(C, H, W) for n in range(NCORES)],
                   axis=0)
    return _host_fixup(nll, x, l)
